# revision 55
# baseline (speedup 1.0000x reference)
"""GQA attention (B=2,S=2048,D=2048,H=16,KV=4,HD=128) + RoPE on 8 TRN2 NeuronCores.

Sharding: core c -> (batch b=c//4, kv-group g=c%4). Each core projects
Q (4 heads), K/V (1 kv head) for its batch from a replicated x^T, applies
RoPE, runs causal flash attention (scores^T layout, no-max softmax --
|scores|<9 so fp32 exp is safe), AllGathers the per-head attention outputs
across the 4-core batch group, and computes a column slice of the output
projection (column-parallel wo).

v2 vs baseline:
- softmax denominator: instead of a ones-matmul per K-block (TensorE), pt
  blocks are accumulated in groups of 8 on DVE and one ones-matmul per
  group accumulates the denominator (saves ~30us TensorE).
- RoPE rotate-half: wq/wk rows are host-permuted into [real(64); imag(64)]
  split layout, so the swap is two partition-offset sb->sb DMA copies
  instead of a permutation matmul.
- causal mask shrunk to the [128,128] diagonal strip.
- DMA ring rebalance: x^T quarters over 4 rings, attention-output staging
  on the gpsimd ring, oproj rhs prefetched on sync/scalar rings, outputs
  on the vector ring.
- end-of-head softmax normalization is emitted one head late so TensorE
  never waits on it.
"""
import numpy as np
import ml_dtypes

import concourse.bass as bass
import concourse.mybir as mybir
import concourse.tile as tile
from concourse import bacc
from concourse.bass import ts
from concourse.bass_utils import run_bass_kernel_spmd

BF = mybir.dt.bfloat16
F32 = mybir.dt.float32
bf16 = ml_dtypes.bfloat16

B, S, D = 2, 2048, 2048
H, KV, HD = 16, 4, 128
NT = 4          # 512-token chunks
ND = 16         # 128-wide D chunks
NH = 4          # heads per core
SCALE = 1.0 / np.sqrt(HD)
RG = [[0, 1, 2, 3], [4, 5, 6, 7]]
MULT = mybir.AluOpType.mult
ADD = mybir.AluOpType.add


def build_nc():
    nc = bacc.Bacc("TRN2", target_bir_lowering=False, debug=False, num_devices=8)
    xt_d = nc.dram_tensor("xt", [D, S], BF, kind="ExternalInput").ap()
    wqkv_d = nc.dram_tensor("wqkvT", [6, 128, 2048], BF, kind="ExternalInput").ap()
    woT_d = nc.dram_tensor("woT", [D, 512], BF, kind="ExternalInput").ap()
    cos_d = nc.dram_tensor("cose", [128, S], BF, kind="ExternalInput").ap()
    sin_d = nc.dram_tensor("sins", [128, S], BF, kind="ExternalInput").ap()
    mask_d = nc.dram_tensor("mask128", [128, 128], BF, kind="ExternalInput").ap()
    ident_d = nc.dram_tensor("ident", [128, 128], BF, kind="ExternalInput").ap()
    onesc_d = nc.dram_tensor("onesc", [128, 128], BF, kind="ExternalInput").ap()
    out_d = nc.dram_tensor("out", [512, S], F32, kind="ExternalOutput").ap()

    xt_r = xt_d.rearrange("(o p) t -> p o t", p=128)      # [128, 16, 2048]
    woT_r = woT_d.rearrange("(o p) m -> p o m", p=128)    # [128, 16, 512]

    with tile.TileContext(nc) as tc:
        with (
            tc.tile_pool(name="consts", bufs=1) as consts,
            tc.tile_pool(name="io", bufs=2) as io,
            tc.tile_pool(name="orhs", bufs=6) as orhs,
            tc.tile_pool(name="work", bufs=2) as work,
            tc.tile_pool(name="psS", bufs=3, space="PSUM") as psS,
            tc.tile_pool(name="psA", bufs=3, space="PSUM") as psA,
            tc.tile_pool(name="psB", bufs=2, space="PSUM") as psB,
            tc.tile_pool(name="dram", bufs=1, space="DRAM") as dram,
        ):
            # ---- persistent SBUF; gpsimd ring order = arrival order.
            w_sb = consts.tile([128, 6, ND, 128], BF, name="w_sb")
            nc.gpsimd.dma_start(
                w_sb[:, 0], wqkv_d[0].rearrange("p (o c) -> p o c", c=128))
            cos_sb = consts.tile([128, S], BF, name="cos_sb")
            nc.gpsimd.dma_start(cos_sb, cos_d)
            sin_sb = consts.tile([128, S], BF, name="sin_sb")
            nc.gpsimd.dma_start(sin_sb, sin_d)
            for m in range(1, 6):
                nc.gpsimd.dma_start(
                    w_sb[:, m], wqkv_d[m].rearrange("p (o c) -> p o c", c=128))
            ident_sb = consts.tile([128, 128], BF, name="ident_sb")
            nc.gpsimd.dma_start(ident_sb, ident_d)
            mask_sb = consts.tile([128, 128], BF, name="mask_sb")
            nc.gpsimd.dma_start(mask_sb, mask_d)
            onesc_sb = consts.tile([128, 128], BF, name="onesc_sb")
            nc.gpsimd.dma_start(onesc_sb, onesc_d)
            woT_sb = consts.tile([128, ND, 512], BF, name="woT_sb")
            nc.gpsimd.dma_start(woT_sb, woT_r)

            qt_sb = consts.tile([128, NH, S], BF, name="qt_sb")   # Q^T, rope'd
            kt_sb = consts.tile([128, S], BF, name="kt_sb")       # K^T, rope'd
            v_sb = consts.tile([128, ND, HD], BF, name="v_sb")    # V [tok, hd]

            # chunks 0-2: one AllGather per chunk (4 heads, identity head
            # order after gather). chunk 3: pair0 + per-head h2/h3 so the
            # final, tail-critical collectives are small.
            ag_in = [dram.tile([512, 512], BF, name=f"agin{i}")
                     for i in range(3)]
            ag_out = [dram.tile([2048, 512], BF, name=f"agout{i}")
                      for i in range(3)]
            ag3_in = [dram.tile([256, 512], BF, name="agin3_p0"),
                      dram.tile([128, 512], BF, name="agin3_h2"),
                      dram.tile([128, 512], BF, name="agin3_h3")]
            ag3_out = [dram.tile([1024, 512], BF, name="agout3_p0"),
                       dram.tile([512, 512], BF, name="agout3_h2"),
                       dram.tile([512, 512], BF, name="agout3_h3")]

            pending = [None]  # deferred end-of-head ops, emitted under cover
            prologue = [None]  # pre-emitted h0 scores/exps for the next attn

            def emit_pending():
                if pending[0] is not None:
                    fn, pending[0] = pending[0], None
                    fn()

            def attn_prologue(qc):
                # h0's first three K-blocks use chunk<qc K and chunk-qc Q
                # (both ready mid-proj) and are full blocks for qc>=1: emit
                # their scores+exp early so the attention start is not gated
                # on the scalar engine draining proj-phase work.
                pro = {}
                for kb in (0, 1, 2):
                    ps_s = psS.tile([128, 512], F32, tag="psS", name="ps_s")
                    nc.tensor.matmul(
                        ps_s, lhsT=kt_sb[:, ts(kb, 128)],
                        rhs=qt_sb[:, 0, ts(qc, 512)], start=True, stop=True)
                    pt = work.tile([128, 512], BF, tag="pt", bufs=6,
                                   name="pt")
                    nc.scalar.activation(
                        pt, ps_s, mybir.ActivationFunctionType.Exp,
                        scale=SCALE)
                    pro[kb] = pt
                prologue[0] = pro

            def proj_chunk(tc_i):
                xt_t = io.tile([128, ND, 512], BF, tag="xt", name="xt_t")
                # chunk 0 splits across both HW DGE rings for fast startup;
                # later chunks stay on sync so the scalar ring is free for
                # rope-swap + attention-output staging DMAs.
                rings = ([nc.sync, nc.scalar, nc.scalar, nc.sync]
                         if tc_i == 0 else [nc.sync] * 4)
                for q in range(4):
                    rings[q].dma_start(xt_t[:, 4 * q:4 * (q + 1), :],
                                       xt_r[:, 4 * q:4 * (q + 1), ts(tc_i, 512)])
                for m in range(6):  # 4 q heads, k, v
                    ps = psA.tile([128, 512], F32, tag="acc", name="ps_proj")
                    for d in range(ND):
                        nc.tensor.matmul(
                            ps, lhsT=w_sb[:, m, d, :], rhs=xt_t[:, d, :],
                            start=(d == 0), stop=(d == ND - 1),
                        )
                    if m == 0:
                        emit_pending()  # prev chunk head-3 fin, under m0 cover
                    if m == 3 and tc_i >= 1:
                        attn_prologue(tc_i)
                    if m < 5:
                        # RoPE (split layout): out = raw*cos + halfswap(raw)*sin
                        raw = work.tile([128, 512], BF, tag="raw", name="raw")
                        nc.scalar.copy(raw, ps)
                        rsw = work.tile([128, 512], BF, tag="rsw", name="rsw")
                        nc.sync.dma_start(rsw[0:64, :], raw[64:128, :])
                        nc.scalar.dma_start(rsw[64:128, :], raw[0:64, :])
                        t1 = work.tile([128, 512], BF, tag="t1", name="t1")
                        nc.vector.tensor_tensor(
                            t1, raw, cos_sb[:, ts(tc_i, 512)], MULT)
                        t2 = work.tile([128, 512], BF, tag="t2", name="t2")
                        nc.vector.tensor_tensor(
                            t2, rsw, sin_sb[:, ts(tc_i, 512)], MULT)
                        dst = (qt_sb[:, m, ts(tc_i, 512)] if m < 4
                               else kt_sb[:, ts(tc_i, 512)])
                        nc.vector.tensor_tensor(dst, t1, t2, ADD)
                    else:
                        # V^T chunk -> bf16 -> transpose to [tok, hd] blocks
                        vraw = work.tile([128, 512], BF, tag="raw", name="vraw")
                        nc.scalar.copy(vraw, ps)
                        pst = psB.tile([128, 4, 128], BF, tag="psB",
                                       name="ps_vT")
                        for j in range(4):
                            nc.tensor.transpose(pst[:, j, :],
                                                vraw[:, ts(j, 128)], ident_sb)
                        nc.vector.tensor_copy(
                            v_sb[:, 4 * tc_i:4 * (tc_i + 1), :], pst)

            def attn_chunk(qc):
                nkb = 4 * qc + 4
                # groups of <=8 k-blocks share one denominator matmul; all
                # groups start at a full-width (o=0) block.
                bounds = [0, min(8, nkb)] + ([nkb] if nkb > 8 else [])
                n_groups = len(bounds) - 1
                for h in range(NH):
                    ps_att = psA.tile([128, 512], F32, tag="acc", name="ps_att")
                    ps_den = psB.tile([128, 512], F32, tag="psB", name="ps_den")
                    sq = {}

                    def emit_scores(kb, h=h):
                        o = max(kb - 4 * qc, 0) * 128
                        ps_s = psS.tile([128, 512], F32, tag="psS", name="ps_s")
                        nc.tensor.matmul(
                            ps_s[:, o:], lhsT=kt_sb[:, ts(kb, 128)],
                            rhs=qt_sb[:, h, 512 * qc + o:512 * (qc + 1)],
                            start=True, stop=True)
                        sq[kb] = (ps_s, o)

                    pro = prologue[0] if h == 0 else None
                    pro_keys = set(pro) if pro is not None else set()
                    if pro is None:
                        emit_scores(0)
                        emit_scores(1)
                    else:
                        prologue[0] = None
                    emit_pending()  # prev head's fin, under scores cover
                    S8 = None
                    gi = 0
                    due = None
                    for kb in range(nkb):
                        if kb in pro_keys:
                            pt = pro.pop(kb)
                            o = 0
                        else:
                            ps_s, o = sq.pop(kb)
                            pt = work.tile([128, 512], BF, tag="pt", bufs=6,
                                           name="pt")
                            nc.scalar.activation(
                                pt[:, o:], ps_s[:, o:],
                                mybir.ActivationFunctionType.Exp, scale=SCALE)
                            if kb - 4 * qc >= 0:  # causal mask, diag strip
                                nc.vector.tensor_tensor(
                                    pt[:, o:o + 128], pt[:, o:o + 128],
                                    mask_sb, MULT)
                        if kb == bounds[gi]:
                            S8 = work.tile([128, 512], BF, tag="spt",
                                           name="spt")
                            nc.vector.tensor_copy(S8[:, o:], pt[:, o:])
                        else:
                            nc.vector.tensor_tensor(
                                S8[:, o:], S8[:, o:], pt[:, o:], ADD)
                        nc.tensor.matmul(
                            ps_att[:, o:], lhsT=v_sb[:, kb, :], rhs=pt[:, o:],
                            start=(kb == 0), stop=(kb == nkb - 1))
                        if kb + 2 < nkb and kb + 2 not in pro_keys:
                            emit_scores(kb + 2)
                        if due is not None and kb == due[0]:
                            nc.tensor.matmul(
                                ps_den, lhsT=onesc_sb, rhs=due[1],
                                start=(due[2] == 0), stop=False)
                            due = None
                        if kb == bounds[gi + 1] - 1 and gi + 1 < n_groups:
                            due = (kb + 2, S8, gi)
                            gi += 1

                    def fin(h=h, qc=qc, ps_att=ps_att, ps_den=ps_den, S8=S8,
                            gi=gi):
                        nc.tensor.matmul(ps_den, lhsT=onesc_sb, rhs=S8,
                                         start=(gi == 0), stop=True)
                        bden = work.tile([128, 512], F32, tag="bden",
                                         name="bden")
                        nc.vector.reciprocal_approx_fast(bden, ps_den)
                        att = work.tile([128, 512], BF, tag="att", name="att")
                        nc.vector.tensor_tensor(att, ps_att, bden, MULT)
                        if qc < 3:
                            nc.scalar.dma_start(
                                ag_in[qc][ts(h, 128), :], att)
                            if h == 3:
                                nc.gpsimd.collective_compute(
                                    "AllGather", mybir.AluOpType.bypass,
                                    replica_groups=RG,
                                    ins=[ag_in[qc][:].opt()],
                                    outs=[ag_out[qc][:].opt()])
                        else:
                            if h < 2:
                                nc.scalar.dma_start(
                                    ag3_in[0][ts(h, 128), :], att)
                            else:
                                nc.scalar.dma_start(ag3_in[h - 1][:, :], att)
                            if h != 0:
                                j = 0 if h == 1 else h - 1
                                nc.gpsimd.collective_compute(
                                    "AllGather", mybir.AluOpType.bypass,
                                    replica_groups=RG,
                                    ins=[ag3_in[j][:].opt()],
                                    outs=[ag3_out[j][:].opt()])

                    pending[0] = fin

            rhs_t = {}

            def emit_rhs(key, n_o, src, ring, wait, tag, bufs):
                # tile_wait_until keeps the scheduler from hoisting these
                # collective-dependent DMAs ahead of urgent loads on the same
                # ring (a waiting descriptor blocks the whole ring).
                with tc.tile_wait_until(wait):
                    r = orhs.tile([128, n_o, 512], BF, tag=tag, bufs=bufs,
                                  name=f"orhs_{key}")
                    ring.dma_start(r, src.rearrange("(o p) t -> p o t", p=128))
                    rhs_t[key] = r

            # chunk-3 oproj accumulation schedule: (rhs key, sub index, head)
            sched3 = ([("3p0", k, [0, 1, 4, 5, 8, 9, 12, 13][k])
                       for k in range(8)]
                      + [("3h2", k, 4 * k + 2) for k in range(4)]
                      + [("3h3", k, 4 * k + 3) for k in range(4)])

            def oproj_chunk(tc_i):
                for j in range(4):
                    ps_o = psA.tile([128, 512], F32, tag="acc", name="ps_o")
                    if tc_i < 3:
                        for c in range(ND):
                            nc.tensor.matmul(
                                ps_o, lhsT=woT_sb[:, c, ts(j, 128)],
                                rhs=rhs_t[tc_i][:, c, :],
                                start=(c == 0), stop=(c == ND - 1))
                    else:
                        for k, (key, sub, head) in enumerate(sched3):
                            nc.tensor.matmul(
                                ps_o, lhsT=woT_sb[:, head, ts(j, 128)],
                                rhs=rhs_t[key][:, sub, :],
                                start=(k == 0), stop=(k == ND - 1))
                    if tc_i == 0 and j == 0:
                        emit_pending()  # chunk-3 head-3 fin, under j0 cover
                        emit_rhs("3h3", 4, ag3_out[2], nc.scalar, 0.215,
                                 "orhsC", 2)
                    o32 = work.tile([128, 512], F32, tag="o32", name="o32")
                    nc.vector.tensor_copy(o32, ps_o)
                    nc.sync.dma_start(out_d[ts(j, 128), ts(tc_i, 512)], o32)

            for i in range(NT):
                proj_chunk(i)
                attn_chunk(i)
            # oproj rhs prefetch (all AllGathers are emitted by now)
            emit_rhs(0, ND, ag_out[0], nc.sync, 0.18, "orhsA", 2)
            emit_rhs(1, ND, ag_out[1], nc.scalar, 0.19, "orhsA", 2)
            emit_rhs(2, ND, ag_out[2], nc.sync, 0.195, "orhsA", 2)
            emit_rhs("3p0", 8, ag3_out[0], nc.scalar, 0.205, "orhsB", 1)
            emit_rhs("3h2", 4, ag3_out[1], nc.sync, 0.21, "orhsC", 2)
            for i in range(NT):
                oproj_chunk(i)

    nc.compile()
    return nc


def make_in_maps(x, freqs_cos, freqs_sin, wq, wk, wv, wo):
    fc = np.asarray(freqs_cos, np.float32)
    fs = np.asarray(freqs_sin, np.float32)
    # split rope layout: partition p<64 = real(freq p), p>=64 = imag(freq p-64)
    cos_exp = np.ascontiguousarray(np.tile(fc.T, (2, 1))).astype(bf16)
    sin_sgn = np.ascontiguousarray(
        np.concatenate([-fs.T, fs.T], axis=0)).astype(bf16)
    mask128 = np.triu(np.ones((128, 128), np.float32)).astype(bf16)
    ident = np.eye(128, dtype=np.float32).astype(bf16)
    onesc = np.ones((128, 128), np.float32).astype(bf16)

    # permute wq/wk rows within each head to the split-pair layout
    perm = np.concatenate([np.arange(0, 128, 2), np.arange(1, 128, 2)])
    wq_p = np.asarray(wq, np.float32).reshape(H, 128, D)[:, perm, :]
    wq_p = wq_p.reshape(H * 128, D)
    wk_p = np.asarray(wk, np.float32).reshape(KV, 128, D)[:, perm, :]
    wk_p = wk_p.reshape(KV * 128, D)

    xt = [np.ascontiguousarray(np.asarray(x[b], np.float32).T).astype(bf16)
          for b in range(B)]
    in_maps = []
    for core in range(8):
        b, g = divmod(core, 4)
        wqkvT = np.concatenate(
            [wq_p[512 * g:512 * (g + 1)].T,
             wk_p[128 * g:128 * (g + 1)].T,
             np.asarray(wv, np.float32)[128 * g:128 * (g + 1)].T], axis=1)
        # m-major SBUF-order blocks: [6][p 128][o*128+c 2048]
        wqkvT = np.ascontiguousarray(
            wqkvT.reshape(16, 128, 768).transpose(2, 1, 0)   # [768 m, 128 p, 16 o]
        )
        wqkvT = np.ascontiguousarray(np.stack(
            [wqkvT[128 * m:128 * (m + 1)].transpose(1, 2, 0).reshape(128, 2048)
             for m in range(6)]))
        woT = np.asarray(wo, np.float32)[512 * g:512 * (g + 1), :].T
        in_maps.append({
            "xt": xt[b],
            "wqkvT": np.ascontiguousarray(wqkvT).astype(bf16),
            "woT": np.ascontiguousarray(woT).astype(bf16),
            "cose": cos_exp,
            "sins": sin_sgn,
            "mask128": mask128,
            "ident": ident,
            "onesc": onesc,
        })
    return in_maps


_NC = None


def get_nc():
    global _NC
    if _NC is None:
        _NC = build_nc()
    return _NC


def assemble_out(results):
    out = np.zeros((B, S, D), np.float32)
    for core in range(8):
        b, g = divmod(core, 4)
        out[b, :, 512 * g:512 * (g + 1)] = results[core]["out"].T
    return out


def kernel(x, freqs_cos, freqs_sin, wq, wk, wv, wo):
    import os
    os.environ.setdefault("BASS_NEVER_TRACE", "1")  # NTFF hook absent headless
    nc = get_nc()
    in_maps = make_in_maps(x, freqs_cos, freqs_sin, wq, wk, wv, wo)
    res = run_bass_kernel_spmd(nc, in_maps, core_ids=list(range(8)))
    return assemble_out(res.results)
